# revision 103
# baseline (speedup 1.0000x reference)
"""Trainium2 Bass kernel for a 2-layer GCN encoder with global mean pool.

Sharding: dst-partition of nodes across 8 NeuronCores (12500 nodes/core,
padded to 12544 slots = 49 cell-blocks of 256, with a load-balancing
permutation of dsts into blocks). Edges live with the core that owns their
dst. Each conv gathers source-node feature rows from a DRAM table via
dma_gather; (block, chunk) cells are sized per-cell (max over cores,
128-slot granularity). The tables are PRE-SCALED by 1/sqrt(deg_src) (x on
the host in bf16, h1 by the conv1 writer), so the per-edge message is the
gathered row and the one-hot "valhot" matrix is a single EQ tensor_scalar
in bf16 (DVE 4x mode, spilling ~1/12th of builds to GpSimd). Messages
aggregate into f32 PSUM via TensorE matmuls (bf16/fp8 stationary x bf16
moving). Self-loop messages are injected from the pre-scaled local fp8
shard with 512B-line DMA loads in (p s) row-group layout and constant
expand matrices E_j (dst == s*p + j); a bf16 GEMM with the layer weight
follows the DVE PSUM->SBUF copy, and per-dst 1/sqrt(deg) scale, bias and
ReLU are a 2-op DVE writer.

Between the convs h1 moves as FP8: conv1 writes an fp8 copy of the scaled
shard in four quarter tensors, and four chunked AllGathers (emitted right
after each quarter's last writer) stream them out WHILE conv1 still runs,
hiding most of the collective behind compute. conv2 gathers 256B row PAIRS
straight from the AllGather output (fp8 [h_2i|h_2i+1] rows); cells are
split by src-row parity so each tile picks its half via a constant lhsT
offset. conv2 itself runs in two passes: pass 1 (self-loop + AG chunks
0..1, whose AllGathers complete before conv1 ends, so nothing camps on an
engine queue) accumulates into PSUM and parks bf16 partials in SBUF;
pass 2 (emitted LAG super-blocks behind) re-injects the partial via a
constant identity matmul and adds chunks 2..3. The self-loop reads the
local fp8 quarter tensors directly (split loads at quarter boundaries).
A tc.no_sync_barrier() between the convs keeps AG-gated work from being
hoisted into conv1's engine queues, where its long semaphore waits would
stall the global sem-range rotation. Per-graph sums ride the same valhot
machinery (batch ids are sorted): each core's sums cover only a W_POOL
graph window, so a small windowed AllGather plus 8 shifted on-core adds
replaces a full-width AllReduce before the two linear heads.

All floating-point math runs on device; the host only prepares integer
index/degree metadata (edge partitioning, packing, int16 gather indices)
plus the dtype conversion/pre-scale of the replicated x table.
"""
import sys

sys.path.insert(0, "/opt/trn_rl_repo")

import numpy as np
import ml_dtypes

BF16NP = ml_dtypes.bfloat16

N = 100000
E = 1600000
G = 256
NCORES = 8
NSHARD = N // NCORES            # 12500 real nodes per core
NPAD = 12544                    # padded shard size (= 49*256 = 98*128)
BLK = 256                       # cell-block width (valhot/psum column range)
NBLK = NPAD // BLK              # 49 cell-blocks per core
NSUB = NPAD // 128              # 98 GEMM sub-blocks per core
CH = 4                          # src chunks (int16 gather index limit)
CSLOT = 1152                    # hard per-cell edge cap (packing feasibility)
PACK_TGT = 1056                 # packer balance target (soft)
W1SZ = 25000                    # conv1 gather window (x table, N rows)
W2SZ = NPAD * 2                 # 25088, conv2 gather window (h1 table)
SBS = [(s * 2, 2) for s in range(24)] + [(48, 1)]   # super-blocks of cell-blocks
F = 128
FO = 64
POOL_EVERY = 18                 # route every POOL_EVERY-th valhot to GpSimd
Q_STARTS = [0, 3200, 6400, 9472]        # h1 AllGather quarter row starts
Q_ROWS = [3200, 3200, 3072, 3072]       # rows per quarter (25/25/24/24 subs)
Q_TAB = [0, 25600, 51200, 75776]        # conv2 table chunk row starts (x8)
W_POOL = 48                             # per-core pooled-graph window width

_CACHE = {}


class Layout:
    """Static per-conv stream layout: per-cell tile counts (max over cores)
    and the derived slot offsets, in SBS-major, chunk-major order. P is the
    number of parity segments per (block, chunk) cell (2 for conv2, whose
    256B pair-gathers need a per-tile src-row parity)."""

    def __init__(self, tiles, P=1):       # tiles: [NBLK, CH*P] int
        self.T = tiles
        self.P = P
        self.off = np.zeros((NBLK, CH * P), np.int64)   # slot offset of cell
        self.goff = np.zeros((len(SBS), CH), np.int64)  # gather slot offset
        self.glen = np.zeros((len(SBS), CH), np.int64)  # gather slot count
        base = 0
        for si, (b0, nb) in enumerate(SBS):
            for k in range(CH):
                self.goff[si, k] = base
                for par in range(P):
                    for bi in range(nb):
                        self.off[b0 + bi, k * P + par] = base
                        base += 128 * tiles[b0 + bi, k * P + par]
                self.glen[si, k] = base - self.goff[si, k]
        self.nslot = int(base)
        self.ntiles = self.nslot // 128

    def key(self):
        return self.T.tobytes() + bytes([self.P])


def _pack_core(deg_tot, cnt8, seed=0):
    """Assign the core's NSHARD dsts to NBLK blocks of <=BLK slots so that no
    (block, chunk) cell exceeds CSLOT edges under either conv's chunking,
    aiming for PACK_TGT. Snake round-robin by degree, then swap-repair."""
    rng = np.random.default_rng(seed)
    order = np.argsort(-deg_tot, kind="stable")
    block_of = np.empty(NSHARD, np.int64)
    seq = np.concatenate([np.arange(NBLK), np.arange(NBLK)[::-1]])
    block_of[order] = np.resize(seq, NSHARD)
    loads = np.zeros((NBLK, 8), np.int64)
    np.add.at(loads, block_of, cnt8)
    for _ in range(9000):
        mx = loads.max()
        if mx <= PACK_TGT:
            return block_of
        b, j = np.unravel_index(np.argmax(loads), loads.shape)
        members = np.where(block_of == b)[0]
        msort = members[np.argsort(-cnt8[members, j])]
        moved = False
        for n in msort[:10]:
            vn = cnt8[n]
            best = None
            for b2 in range(NBLK):
                if b2 == b:
                    continue
                mem2 = np.where(block_of == b2)[0]
                v2 = cnt8[mem2]
                nb = loads[b] - vn[None, :] + v2
                nb2 = loads[b2] + vn[None, :] - v2
                s = np.maximum(nb.max(axis=1), nb2.max(axis=1))
                k = int(np.argmin(s))
                if best is None or s[k] < best[0]:
                    best = (s[k], mem2[k], b2)
            if best is not None and best[0] < mx:
                _, n2, b2 = best
                block_of[n], block_of[n2] = b2, b
                loads[b] += cnt8[n2] - vn
                loads[b2] += vn - cnt8[n2]
                moved = True
                break
        if not moved:
            n = rng.choice(members)
            b2 = int(rng.integers(NBLK))
            if b2 == b:
                continue
            mem2 = np.where(block_of == b2)[0]
            n2 = rng.choice(mem2)
            block_of[n], block_of[n2] = b2, b
            loads[b] += cnt8[n2] - cnt8[n]
            loads[b2] += cnt8[n] - cnt8[n2]
    if loads.max() <= CSLOT:
        return block_of
    raise RuntimeError("cell packing failed; raise CSLOT")


def _host_prep(x, edge_index, batch):
    srcF = edge_index[0].astype(np.int64)
    dstF = edge_index[1].astype(np.int64)
    # degrees include the self-loop (+1); self-loop messages are injected
    # on-device from the local shard, not via the gather stream
    deg = np.bincount(dstF, minlength=N).astype(np.int64) + 1

    owner_e = dstF // NSHARD
    chunk1 = srcF // W1SZ                 # conv1: raw x row windows

    # conv2 chunk of a src node depends on its position in the packed shard
    # (quarter of the owner's shard), which is itself an output of packing.
    # Packing uses a preliminary chunk2 estimate from the node id (quarters
    # of the owner's raw range) — close enough for load balancing; the exact
    # chunk2 is recomputed from tablerow afterwards.
    chunk2_pre = (srcF % NSHARD) * 4 // NSHARD

    # --- pack every core's dsts into blocks ---------------------------------
    block_of_g = np.empty(N, np.int64)
    slot_of_g = np.empty(N, np.int64)
    for c in range(NCORES):
        base = c * NSHARD
        m = owner_e == c
        ed = dstF[m] - base
        c1 = np.bincount(ed * CH + chunk1[m], minlength=NSHARD * CH)
        c2 = np.bincount(ed * CH + chunk2_pre[m], minlength=NSHARD * CH)
        cnt8 = np.concatenate(
            [c1.reshape(NSHARD, CH), c2.reshape(NSHARD, CH)], axis=1
        )
        blk = _pack_core(deg[base : base + NSHARD], cnt8)
        block_of_g[base : base + NSHARD] = blk
        # slot within block: stable order of nodes per block
        o = np.argsort(blk, kind="stable")
        r = np.empty(NSHARD, np.int64)
        r[o] = np.arange(NSHARD) - np.searchsorted(blk[o], blk[o])
        slot_of_g[base : base + NSHARD] = r

    node_owner = np.arange(N) // NSHARD
    shardrow = block_of_g * BLK + slot_of_g                      # per node
    tablerow = node_owner * NPAD + shardrow                      # per node
    # conv2 gather table is chunk-interleaved: [chunk(4), owner(8), rows_c]
    # with quarter boundaries on writer sub-blocks (25/25/24/24 sub-blocks)
    qs = np.array(Q_STARTS + [NPAD], np.int64)
    sr = shardrow[srcF]
    chunk2 = np.searchsorted(qs[1:], sr, side="right")
    tab2row = (node_owner[srcF] * np.array(Q_ROWS, np.int64)[chunk2]
               + sr - qs[chunk2])

    dinv = (1.0 / np.sqrt(np.maximum(deg, 1))).astype(np.float32)
    x_scaled = (np.asarray(x, np.float32) * dinv[:, None]).astype(BF16NP)
    dstslot = tablerow % BLK              # position of a dst inside its block

    # --- per-cell loads -> static per-cell tile counts (max over cores) -----
    cells = [np.empty((NCORES, NBLK * CH * (conv + 1)), np.int64)
             for conv in range(2)]
    for c in range(NCORES):
        m = owner_e == c
        eblk = block_of_g[dstF[m]]
        par2 = tab2row[m] % 2
        for conv, key in enumerate(
            [eblk * CH + chunk1[m], (eblk * CH + chunk2[m]) * 2 + par2]
        ):
            cells[conv][c] = np.bincount(
                key, minlength=NBLK * CH * (conv + 1)
            )
    layouts = []
    for conv in range(2):
        P = conv + 1
        mx = cells[conv].max(axis=0).reshape(NBLK, CH * P)
        if mx.max() > CSLOT:
            raise RuntimeError("cell overflow; raise CSLOT")
        layouts.append(Layout(-(-mx // 128), P))

    # per-core pooled-graph windows: batch is sorted and the dst shard is a
    # contiguous node range, so core c's pool sums touch only graphs
    # [pool_gs[c], pool_gs[c] + W_POOL)
    pool_gs = [int(batch[c * NSHARD]) for c in range(NCORES)]
    for c in range(NCORES):
        w = int(batch[(c + 1) * NSHARD - 1]) - pool_gs[c] + 1
        assert w <= W_POOL, f"pool window {w} > {W_POOL}"

    per_core = []
    for c in range(NCORES):
        base = c * NSHARD
        m = owner_e == c
        es, ed = srcF[m], dstF[m]
        ec1, ec2 = chunk1[m], chunk2[m]
        eblk = block_of_g[ed]

        core = {}
        for conv, (cell, idxval) in enumerate(
            [
                (eblk * CH + ec1, es % W1SZ),
                ((eblk * CH + ec2) * 2 + tab2row[m] % 2, tab2row[m] // 2),
            ]
        ):
            lay = layouts[conv]
            ncell = NBLK * CH * lay.P
            o = np.argsort(cell, kind="stable")
            cell_s = cell[o]
            cnt = np.bincount(cell_s, minlength=ncell)
            starts = np.zeros(ncell, np.int64)
            starts[1:] = np.cumsum(cnt)[:-1]
            rank = np.arange(len(cell_s)) - starts[cell_s]
            pos = lay.off.reshape(-1)[cell_s] + rank

            idxv = np.zeros(lay.nslot, np.int16)
            dlv = np.full(lay.nslot, -1.0, np.float32)
            idxv[pos] = idxval[o].astype(np.int16)
            dlv[pos] = dstslot[ed[o]].astype(np.float32)

            wrapped = np.ascontiguousarray(idxv.reshape(-1, 16).T)  # [16, nslot/16]
            core[f"idx{conv + 1}"] = np.tile(wrapped, (8, 1))       # [128, nslot/16]
            core[f"dl{conv + 1}"] = np.ascontiguousarray(
                dlv.reshape(-1, 128).T
            )                                                        # [128, ntiles]

        # per-slot node metadata in [slot%128, slot//128] layout
        nodes = np.arange(base, base + NSHARD)
        slotidx = block_of_g[nodes] * BLK + slot_of_g[nodes]
        degd = np.ones(NPAD, np.float32)
        degd[slotidx] = deg[nodes].astype(np.float32)
        blv = np.full(NPAD, -1.0, np.float32)
        blv[slotidx] = (batch[nodes] - pool_gs[c]).astype(np.float32)
        core["degd"] = np.ascontiguousarray(degd.reshape(NSUB, 128).T)
        core["bl"] = np.ascontiguousarray(blv.reshape(NSUB, 128).T)
        # permuted, pre-scaled local x shard (slot order) for the self-loop
        xp = np.zeros((NPAD, F), BF16NP)
        xp[slotidx] = x_scaled[nodes]
        core["x_perm"] = xp
        per_core.append(core)

    return per_core, x_scaled, layouts, pool_gs


def _build_bass(layouts, pool_gs):
    import os
    from concourse import bacc, tile, bass
    import concourse.mybir as mybir

    mode = os.environ.get("KBUILD_MODE", "full")

    F32 = mybir.dt.float32
    BF16 = mybir.dt.bfloat16
    FP8 = mybir.dt.float8e4
    I16 = mybir.dt.int16
    EQ = mybir.AluOpType.is_equal
    MULT = mybir.AluOpType.mult
    ADD = mybir.AluOpType.add
    MAX = mybir.AluOpType.max
    AF = mybir.ActivationFunctionType

    nc = bacc.Bacc("TRN2", target_bir_lowering=False, debug=False,
                   num_devices=NCORES)

    x_tab = nc.dram_tensor("x_tab", [N, F], BF16, kind="ExternalInput")
    x_perm_d = nc.dram_tensor("x_perm", [NPAD, F], BF16, kind="ExternalInput")
    idx_d = [nc.dram_tensor(f"idx{i+1}", [128, layouts[i].nslot // 16], I16,
                            kind="ExternalInput") for i in range(2)]
    dl_d = [nc.dram_tensor(f"dl{i+1}", [128, layouts[i].ntiles], F32,
                           kind="ExternalInput") for i in range(2)]
    iota_d = nc.dram_tensor("iota", [128, 512], BF16, kind="ExternalInput")
    pexp_d = nc.dram_tensor("pexp", [128, 7], F32, kind="ExternalInput")
    degd_d = nc.dram_tensor("degd", [128, NSUB], F32, kind="ExternalInput")
    bl_d = nc.dram_tensor("bl", [128, NSUB], F32, kind="ExternalInput")
    w_d = [nc.dram_tensor(f"w{i+1}", [F, F], BF16, kind="ExternalInput")
           for i in range(2)]
    bbc_d = [nc.dram_tensor(f"b{i+1}bc", [128, F], F32, kind="ExternalInput")
             for i in range(2)]
    wmu_d = nc.dram_tensor("wmu", [F, FO], F32, kind="ExternalInput")
    wlv_d = nc.dram_tensor("wlv", [F, FO], F32, kind="ExternalInput")
    bmu_d = nc.dram_tensor("bmubc", [128, FO], F32, kind="ExternalInput")
    blv_d = nc.dram_tensor("blvbc", [128, FO], F32, kind="ExternalInput")
    cnt_d = nc.dram_tensor("cnt", [128, 2], F32, kind="ExternalInput")

    mu_o = nc.dram_tensor("mu", [G, FO], F32, kind="ExternalOutput")
    lv_o = nc.dram_tensor("lv", [G, FO], F32, kind="ExternalOutput")
    h1_o = (nc.dram_tensor("h1", [NPAD, F], BF16, kind="ExternalOutput")
            if "conv1only" in mode else None)

    with tile.TileContext(nc) as tc:
        with (
            tc.tile_pool(name="const", bufs=1) as cp,
            tc.tile_pool(name="stream", bufs=4) as sp,
            tc.tile_pool(name="work", bufs=8) as wp,
            tc.tile_pool(name="accp", bufs=25) as ap_,
            tc.tile_pool(name="idxp", bufs=1) as ip,
            tc.tile_pool(name="psum", bufs=2, space="PSUM") as pp,
            tc.tile_pool(name="psum3", bufs=3, space="PSUM") as pp3,
            tc.tile_pool(name="psum1", bufs=1, space="PSUM") as pp1,
            tc.tile_pool(name="dram", bufs=1, space="DRAM") as dp,
        ):
            # ---- conv1 index stream first: its load gates the very first
            # gathers, so it must not queue behind the constant loads ------
            w1 = layouts[0].nslot // 16
            idxfull1 = ip.tile([128, w1], I16, tag="idxfull0")
            cut1 = (w1 // 4) & ~127
            nc.sync.dma_start(idxfull1[:, :cut1], idx_d[0][:, :cut1])
            nc.sync.dma_start(idxfull1[:, cut1:], idx_d[0][:, cut1:])

            # ---- constants -------------------------------------------------
            iotaw = cp.tile([128, 512], BF16, tag="iotaw")
            nc.sync.dma_start(iotaw[:], iota_d[:])
            iotab = iotaw[:, :256]
            pexp = cp.tile([128, 7], F32, tag="pexp")
            nc.sync.dma_start(pexp[:], pexp_d[:])
            zerosb = cp.tile([128, 512], BF16, tag="zerosb")
            nc.vector.memset(zerosb[:], 0.0)
            # constant expand matrices: E4[j][p, d] = (d == 4p+j) over 512,
            # E2[j][p, d] = (d == 2p+j) over 256 (self-loop injection)
            ident = cp.tile([128, 128], BF16, tag="ident")
            e4 = [cp.tile([128, 512], BF16, tag=f"e4_{j}", name=f"e4_{j}")
                  for j in range(4)]
            e2 = [cp.tile([128, 256], BF16, tag=f"e2_{j}", name=f"e2_{j}")
                  for j in range(2)]
            for j in range(4):
                nc.vector.tensor_scalar(e4[j][:], iotaw[:], pexp[:, j : j + 1],
                                        None, EQ)
            for j in range(2):
                nc.vector.tensor_scalar(e2[j][:], iotab, pexp[:, 4 + j : 5 + j],
                                        None, EQ)
            nc.vector.tensor_scalar(ident[:], iotaw[:, :128], pexp[:, 6:7],
                                    None, EQ)
            w_sb = [cp.tile([F, F], BF16, tag=f"w{i}", name=f"w{i}")
                    for i in range(2)]
            bbc_sb = [cp.tile([128, F], F32, tag=f"bbc{i}", name=f"bbc{i}")
                      for i in range(2)]
            for i in range(2):
                nc.sync.dma_start(w_sb[i][:], w_d[i][:])
                nc.sync.dma_start(bbc_sb[i][:], bbc_d[i][:])
            wmu = cp.tile([F, FO], F32, tag="wmu")
            wlv = cp.tile([F, FO], F32, tag="wlv")
            bmu = cp.tile([128, FO], F32, tag="bmu")
            blv = cp.tile([128, FO], F32, tag="blv")
            for t, d in [(wmu, wmu_d), (wlv, wlv_d), (bmu, bmu_d), (blv, blv_d)]:
                nc.sync.dma_start(t[:], d[:])

            bl_sb = cp.tile([128, NSUB], F32, tag="bl")
            nc.sync.dma_start(bl_sb[:], bl_d[:])

            # dinv over the dst shard: 1/sqrt(max(deg,1))
            degd = cp.tile([128, NSUB], F32, tag="degd")
            nc.sync.dma_start(degd[:], degd_d[:])
            dinvd = cp.tile([128, NSUB], F32, tag="dinvd")
            nc.vector.tensor_scalar(degd[:], degd[:], 1.0, None, MAX)
            nc.scalar.activation(degd[:], degd[:], AF.Sqrt)
            nc.vector.reciprocal(dinvd[:], degd[:])

            dl_sb = []
            for i in range(2):
                dl = cp.tile([128, layouts[i].ntiles], F32, tag=f"dl{i}",
                             name=f"dl{i}")
                nc.sync.dma_start(dl[:], dl_d[i][:])
                dl_sb.append(dl)

            # cnt -> 1/max(cnt,1)
            cnt = cp.tile([128, 2], F32, tag="cnt")
            nc.sync.dma_start(cnt[:], cnt_d[:])
            rcnt = cp.tile([128, 2], F32, tag="rcnt")
            nc.vector.tensor_scalar(cnt[:], cnt[:], 1.0, None, MAX)
            nc.vector.reciprocal(rcnt[:], cnt[:])

            # ---- DRAM intermediates ---------------------------------------
            # fp8 shard quarters: separate tensors so a quarter's AllGather
            # read can't fence the next quarter's writer DMAs
            h1_fp8_q = [dp.tile([Q_ROWS[c], F], FP8, name=f"h1q{c}")
                        for c in range(4)]
            # gathered fp8 / row-duplicated fp8 gather table, one tensor per
            # AG chunk so the whole-tensor dependency tracking of collective
            # operands can't fence chunk c+1's producers on chunk c's
            # consumers. h1p rows are [h_r | h_r] so the 256B-elem dma_gather
            # stride constraint is met without an ALU expansion pass.
            # gathered fp8 shards, declared as row PAIRS [h_2i | h_2i+1] so
            # conv2's 256B-elem dma_gather meets the stride constraint; the
            # per-tile src-row parity picks the half at matmul time
            h8_q = [dp.tile([NCORES * Q_ROWS[c] // 2, 2 * F], FP8,
                            name=f"h8q{c}") for c in range(4)]
            sums_in = dp.tile([128, W_POOL], F32)
            sums_g = dp.tile([NCORES * 128, W_POOL], F32)

            pool_ps = (None if "conv1only" in mode
                       else pp1.tile([128, W_POOL], F32, tag="pool",
                                     name="pool_ps"))
            vh_count = [0]

            def load_idx(conv):
                # split the load so the first gathers only wait on the
                # leading quarter of the index stream
                lay = layouts[conv]
                w = lay.nslot // 16
                idxfull = ip.tile([128, w], I16, tag=f"idxfull{conv}")
                cut = (w // 4) & ~127
                nc.sync.dma_start(idxfull[:, :cut], idx_d[conv][:, :cut])
                nc.sync.dma_start(idxfull[:, cut:], idx_d[conv][:, cut:])
                return idxfull

            def run_conv(conv, table, windows, selftab, writer,
                         after_writer=None, two_pass=False, idxfull=None):
                lay = layouts[conv]
                dls = dl_sb[conv]
                if idxfull is None:
                    idxfull = load_idx(conv)
                mf = 2 * F if conv == 1 else F   # fp8 pair rows for conv2
                mdt = FP8 if conv == 1 else BF16
                P = lay.P

                def emit_self(agg, b0, nb):
                    # self-loop from the pre-scaled local shard: partition p
                    # carries rows s*p+j of the nb*256 group; constant
                    # expand matrices place them on the psum diagonal.
                    # conv1 reads the bf16 x_perm table; conv2 reads the
                    # local fp8 quarter tensors (split loads at quarter
                    # boundaries, which are 128-row multiples so the
                    # affected partition ranges stay contiguous).
                    s = nb * 2
                    sdt = BF16 if conv == 0 else FP8
                    xl = sp.tile([128, s * F], sdt, tag=f"xl{conv}")
                    r0, rows = b0 * BLK, nb * BLK
                    if conv == 0:
                        nc.scalar.dma_start(
                            xl[:],
                            selftab[r0 : r0 + rows, :].rearrange(
                                "(p s) f -> p (s f)", s=s
                            ),
                        )
                    else:
                        qends = Q_STARTS[1:] + [NPAD]
                        for q in range(4):
                            a = max(r0, Q_STARTS[q])
                            bnd = min(r0 + rows, qends[q])
                            if a >= bnd:
                                continue
                            pa = (a - r0) // s
                            pb = (bnd - r0) // s
                            nc.scalar.dma_start(
                                xl[pa:pb, :],
                                selftab[q][a - Q_STARTS[q] :
                                           bnd - Q_STARTS[q], :].rearrange(
                                    "(p s) f -> p (s f)", s=s
                                ),
                            )
                    es = e4 if nb == 2 else e2
                    for j in range(s):
                        nc.tensor.matmul(
                            agg[:, : nb * 256],
                            xl[:, j * F : (j + 1) * F], es[j][:],
                            start=False, stop=False,
                        )

                def emit_cells(agg, si, b0, nb, ks):
                    for k in ks:
                        goff = int(lay.goff[si, k])
                        clen = int(lay.glen[si, k])
                        if clen == 0:
                            continue
                        wtab, ws, wn = windows[k]
                        msg = sp.tile([128, clen // 128, mf], mdt,
                                      tag=f"msg{conv}")
                        nc.gpsimd.dma_gather(
                            msg[:, : clen // 128, :],
                            wtab[ws : ws + wn, :],
                            idxfull[:, goff // 16 : (goff + clen) // 16],
                            clen, clen, mf, elem_step=mf,
                            single_packet=False,
                        )
                        m2 = msg.rearrange("p t f -> p (t f)")
                        for par in range(P):
                            for bi in range(nb):
                                kc = k * P + par
                                for t in range(int(lay.T[b0 + bi, kc])):
                                    tl = int(lay.off[b0 + bi, kc] - goff) // 128 + t
                                    col = int(lay.off[b0 + bi, kc]) // 128 + t
                                    vh = wp.tile([128, 256], BF16,
                                                 tag=f"vh{conv}")
                                    eng = (nc.gpsimd
                                           if vh_count[0] % POOL_EVERY == POOL_EVERY - 1
                                           else nc.vector)
                                    vh_count[0] += 1
                                    eng.tensor_scalar(
                                        vh[:], iotab,
                                        dls[:, col : col + 1], None, EQ,
                                    )
                                    nc.tensor.matmul(
                                        agg[:, bi * 256 : (bi + 1) * 256],
                                        m2[:, tl * mf + par * F :
                                           tl * mf + par * F + 128],
                                        vh[:],
                                        start=False,
                                        stop=False,
                                    )

                def open_agg():
                    agg = pp3.tile([128, 512], F32, tag="agg")
                    # HW: start=True clears has_written for the WHOLE psum
                    # bank — exactly one full-width start matmul per bank.
                    nc.tensor.matmul(agg[:], zerosb[:, :128], zerosb[:],
                                     start=True, stop=False)
                    return agg

                def emit_tail(agg, b0, nb):
                    # close the accumulation group
                    nc.tensor.matmul(agg[:, :128], zerosb[:, :128],
                                     zerosb[:, :128], start=False, stop=True)
                    aggT = wp.tile([128, 512], BF16, tag="aggT")
                    nc.scalar.activation(
                        aggT[:, : nb * 256], agg[:, : nb * 256], AF.Copy
                    )
                    for sub in range(nb * 2):
                        b128 = b0 * 2 + sub
                        gm = pp.tile([128, F], F32, tag="gemm")
                        nc.tensor.matmul(
                            gm[:], aggT[:, sub * 128 : (sub + 1) * 128],
                            w_sb[conv][:], start=True, stop=True,
                        )
                        writer(b128, gm)
                        if after_writer is not None:
                            after_writer(b128)

                if not two_pass:
                    for si, (b0, nb) in enumerate(SBS):
                        agg = open_agg()
                        emit_self(agg, b0, nb)
                        emit_cells(agg, si, b0, nb, range(CH))
                        emit_tail(agg, b0, nb)
                else:
                    # pass 1: self + chunks 0..CH-2 accumulate while the
                    # last AllGather chunk is still in flight; park the
                    # partials in SBUF. Pass-2 sbs are emitted LAG super-
                    # blocks behind pass 1 so their final-chunk work
                    # overlaps the still-DMA-bound pass-1 stretch without
                    # camping on the last AllGather.
                    accs = []
                    LAG = 18

                    def emit_p2(si):
                        b0, nb = SBS[si]
                        agg = open_agg()
                        nc.tensor.matmul(
                            agg[:, : nb * 256], ident[:],
                            accs[si][:, : nb * 256],
                            start=False, stop=False,
                        )
                        emit_cells(agg, si, b0, nb, [CH - 2, CH - 1])
                        emit_tail(agg, b0, nb)

                    for si, (b0, nb) in enumerate(SBS):
                        agg = open_agg()
                        emit_self(agg, b0, nb)
                        emit_cells(agg, si, b0, nb, range(CH - 2))
                        nc.tensor.matmul(agg[:, :128], zerosb[:, :128],
                                         zerosb[:, :128], start=False,
                                         stop=True)
                        acc = ap_.tile([128, 512], BF16, tag="acc")
                        nc.scalar.activation(
                            acc[:, : nb * 256], agg[:, : nb * 256], AF.Copy
                        )
                        accs.append(acc)
                        if si >= LAG:
                            emit_p2(si - LAG)
                    for si in range(len(SBS) - LAG, len(SBS)):
                        emit_p2(si)

            def w_conv1(b, gm):
                t = wp.tile([128, F], F32, tag="ht")
                nc.vector.scalar_tensor_tensor(
                    t[:], gm[:], dinvd[:, b : b + 1], bbc_sb[0][:], MULT, ADD,
                )
                # relu, then fold in the 1/sqrt(deg_dst) pre-scale for the
                # conv2 gather table (dinv > 0 commutes with relu)
                h8 = wp.tile([128, F], FP8, tag="h8")
                nc.vector.tensor_scalar(
                    h8[:], t[:], 0.0, dinvd[:, b : b + 1], MAX, MULT,
                )
                qc = (0 if b < 25 else 1 if b < 50 else 2 if b < 74 else 3)
                r = b * 128 - Q_STARTS[qc]
                nc.sync.dma_start(h1_fp8_q[qc][r : r + 128, :], h8[:])

            def w_conv2(b, gm):
                t = wp.tile([128, F], F32, tag="h2t")
                nc.vector.scalar_tensor_tensor(
                    t[:], gm[:], dinvd[:, b : b + 1], bbc_sb[1][:], MULT, ADD,
                )
                h = wp.tile([128, F], BF16, tag="h2")
                nc.vector.tensor_scalar(h[:], t[:], 0.0, None, MAX)
                ph = wp.tile([128, W_POOL], BF16, tag="ph")
                nc.any.tensor_scalar(
                    ph[:], iotab[:, :W_POOL], bl_sb[:, b : b + 1], None, EQ,
                )
                nc.tensor.matmul(
                    pool_ps[:], h[:], ph[:],
                    start=(b == 0), stop=(b == NSUB - 1),
                )

            def emit_ag(c):
                # chunked fp8 AllGather: quarter-shards stream out while
                # conv1 is still producing later quarters (emitted right
                # after the quarter's last writer so the collective queue
                # slot becomes ready in order)
                nc.gpsimd.collective_compute(
                    "AllGather", mybir.AluOpType.bypass,
                    replica_groups=[list(range(NCORES))],
                    ins=[h1_fp8_q[c].opt()],
                    outs=[h8_q[c].opt()],
                )

            AG_AT = {24: 0, 49: 1, 73: 2, 97: 3}

            def conv1_after_writer(b128):
                if b128 in AG_AT and "conv1only" not in mode:
                    emit_ag(AG_AT[b128])

            run_conv(0, x_tab,
                     [(x_tab, k * W1SZ, W1SZ) for k in range(CH)],
                     x_perm_d, w_conv1, after_writer=conv1_after_writer,
                     idxfull=idxfull1)

            if "conv1only" not in mode:
                idxfull2 = load_idx(1)
                # scheduler fence: nothing after this point (the AG-gated
                # duplication copies in particular) may be hoisted into
                # conv1's engine queues, where their long semaphore waits
                # would stall the global sem-range rotation
                tc.no_sync_barrier()
                # conv2's table reads depend on the AllGathers
                # automatically: under TileContext every DMA AP lowers
                # symbolically, so the scheduler sees the RAW chain
                # h1_fp8_q -> h8_q.
                run_conv(1, None,
                         [(h8_q[k], 0, NCORES * Q_ROWS[k] // 2)
                          for k in range(CH)],
                         h1_fp8_q, w_conv2, two_pass=True,
                         idxfull=idxfull2)

            # ---- pooling sums AllReduce + heads ---------------------------
            if "conv1only" in mode:
                dummy = wp.tile([128, FO], F32, tag="headsb")
                nc.vector.memset(dummy[:], 0.0)
                for j in range(2):
                    for out_d in (mu_o, lv_o):
                        nc.sync.dma_start(
                            out_d[j * 128 : (j + 1) * 128, :], dummy[:])
                zf = wp.tile([128, F], BF16, tag="hzero")
                nc.vector.memset(zf[:], 0.0)
                for bb in range(NSUB):
                    nc.sync.dma_start(h1_o[bb * 128 : (bb + 1) * 128, :],
                                      zf[:])
            else:
                pool_sb = wp.tile([128, W_POOL], F32, tag="poolsb")
                nc.vector.tensor_copy(pool_sb[:], pool_ps[:])
                nc.sync.dma_start(sums_in[:], pool_sb[:])
                # each core's sums only cover its W_POOL-graph window, so an
                # AllGather of the windows (+8 shifted adds) replaces the
                # 1.875x-penalized full-width AllReduce
                nc.gpsimd.collective_compute(
                    "AllGather", mybir.AluOpType.bypass,
                    replica_groups=[list(range(NCORES))],
                    ins=[sums_in.opt()], outs=[sums_g.opt()],
                )
                gth = wp.tile([128, NCORES, W_POOL], F32, tag="gth")
                nc.sync.dma_start(
                    gth[:],
                    sums_g[:].rearrange("(c p) w -> p c w", c=NCORES),
                )
                sums_sb = wp.tile([128, 256], F32, tag="sums")
                nc.vector.memset(sums_sb[:], 0.0)
                for c in range(NCORES):
                    w = min(W_POOL, 256 - pool_gs[c])
                    nc.vector.tensor_tensor(
                        sums_sb[:, pool_gs[c] : pool_gs[c] + w],
                        sums_sb[:, pool_gs[c] : pool_gs[c] + w],
                        gth[:, c, :w], ADD,
                    )
                for j in range(2):
                    for wt, bt, out_d in [(wmu, bmu, mu_o), (wlv, blv, lv_o)]:
                        hp = pp.tile([128, FO], F32, tag="head")
                        nc.tensor.matmul(
                            hp[:], sums_sb[:, j * 128 : (j + 1) * 128], wt[:],
                            start=True, stop=True,
                        )
                        hs = wp.tile([128, FO], F32, tag="headsb")
                        nc.vector.scalar_tensor_tensor(
                            hs[:], hp[:], rcnt[:, j : j + 1], bt[:], MULT, ADD,
                        )
                        nc.sync.dma_start(
                            out_d[j * 128 : (j + 1) * 128, :], hs[:])

    nc.compile()
    return nc


def kernel(x, edge_index, batch, W1, b1, W2, b2, W_mu, b_mu, W_lv, b_lv):
    from concourse import bass_utils

    x = np.asarray(x, dtype=np.float32)
    edge_index = np.asarray(edge_index)
    batch = np.asarray(batch)

    per_core, x_scaled, layouts, pool_gs = _host_prep(x, edge_index, batch)

    iota = np.broadcast_to(
        np.arange(512, dtype=np.float32), (128, 512)
    ).astype(BF16NP)
    p = np.arange(128, dtype=np.float32)
    pexp = np.stack(
        [4 * p, 4 * p + 1, 4 * p + 2, 4 * p + 3, 2 * p, 2 * p + 1, p], axis=1
    )
    cnts = np.bincount(np.asarray(batch, np.int64), minlength=G).astype(np.float32)
    cnt_arr = np.ascontiguousarray(cnts.reshape(2, 128).T)
    shared = dict(
        x_tab=x_scaled,
        iota=iota,
        pexp=pexp,
        w1=np.asarray(W1, np.float32).astype(BF16NP),
        w2=np.asarray(W2, np.float32).astype(BF16NP),
        b1bc=np.broadcast_to(np.asarray(b1, np.float32), (128, F)).copy(),
        b2bc=np.broadcast_to(np.asarray(b2, np.float32), (128, F)).copy(),
        wmu=np.asarray(W_mu, np.float32), wlv=np.asarray(W_lv, np.float32),
        bmubc=np.broadcast_to(np.asarray(b_mu, np.float32), (128, FO)).copy(),
        blvbc=np.broadcast_to(np.asarray(b_lv, np.float32), (128, FO)).copy(),
        cnt=cnt_arr,
    )
    in_maps = [dict(shared, **pc) for pc in per_core]

    key = (layouts[0].key(), layouts[1].key(), tuple(pool_gs))
    if _CACHE.get("key") != key:
        _CACHE["nc"] = _build_bass(layouts, pool_gs)
        _CACHE["key"] = key
    nc = _CACHE["nc"]

    import os as _os
    res = bass_utils.run_bass_kernel_spmd(
        nc, in_maps, core_ids=list(range(NCORES)),
        trace=_os.environ.get("KTRACE") == "1",
    )
    _CACHE["last_res"] = res
    r0 = res.results[0]
    return (r0["mu"].copy(), r0["lv"].copy())
